# revision 65
# baseline (speedup 1.0000x reference)
"""Trainium2 Bass kernel for Coo2FulSimple (periodic pairwise squared
distances + cutoff adjacency mask).

Contract: kernel(**inputs) takes the FULL unsharded inputs (numpy) and
returns the FULL outputs (out [B,N,N,S] f32, mask [B,N,N,S] bool).

Key structure (validated bit-exact in numpy against the reference):
  * Exact mirror symmetry: sod[b,i,j,s] == sod[b,j,i,26-s] bitwise
    (IEEE fl() is sign-symmetric and t[26-s] == -t[s] exactly), so the
    device computes only half the pairs: j = (i + r) mod N, r in
    [1, N/2]. The host scatters the slab to both (i,j,s) and
    (j,i,26-s); the diagonal (i==j) is exactly zero in both outputs.
  * Positions are replicated to SBUF partition p pre-shifted by the
    row index ("skew"), so j = i + r becomes a plain free-axis index.
  * Device chain, matching the f32 reference rounding:
      W_ck = Square(-pos_j + fl(pos_i + t_ck))   (ACT, fused bias)
      P    = W0_k0 + W1_k1                        (DVE or Pool)
      sod  = P + W2_k2                            (DVE or Pool)
    The full f32 sod slab is DMAed out; the cutoff select
    (where(sod <= rc^2, sod, 0)) runs on the host during the gather
    pass it already performs (replacing a dtype-conversion pass), so
    the output is the exact f32 sod and mask == (out > 0) exactly.
  * The kernel is DMA-roofline-bound: engines (~14 us each) feed a
    saturated ~21 us f32 output stream; chunk sizes ramp up from 8 r
    so the stream starts as early as possible, and a virtual-clock
    pre-scheduler orders each engine queue to avoid head-of-line
    stalls.

Sharding: 16 slabs = (batch b in 4) x (i-tile in 4 of 128 rows), two
slabs per core across 8 NeuronCores.
"""

import os
from contextlib import ExitStack

import numpy as np

B, N, S = 4, 512, 27
NCORES = 8
IT = 128          # i-tile size == SBUF partitions
R = 256           # r-extent (j = i + 1 + x, x in [0, R))
UNITS = 2         # i-tiles per core
RC2 = 36.0

SKW = 3 * R                      # skew floats per unit per partition
UW = SKW + 9                     # per-unit cst block: biases + skew
CW = UNITS * UW                  # cst width
RL = 64                          # r-ladder granularity for unit 0
# W r-ladder pieces per unit: unit 0 fine-grained (its delivery gates the
# pipeline start), unit 1 coarse (never critical)
PIECES_U = [((0, 64), (64, 128), (128, 192), (192, 256)),
            ((0, 64), (64, 256))]

_CACHE = {}


def _build_program():
    import concourse.bacc as bacc
    import concourse.mybir as mybir
    import concourse.tile as tile

    f32 = mybir.dt.float32
    f16 = mybir.dt.float16
    SQUARE = mybir.ActivationFunctionType.Square
    ADD = mybir.AluOpType.add
    MULT = mybir.AluOpType.mult
    IS_LE = mybir.AluOpType.is_le

    nc = bacc.Bacc(
        "TRN2", target_bir_lowering=False, debug=False, num_devices=NCORES
    )

    cst = nc.dram_tensor("cst", [IT, CW], f32, kind="ExternalInput").ap()
    outv = nc.dram_tensor("outv", [UNITS, IT, R, S], f32, kind="ExternalOutput").ap()

    # (start, end, owner): owner computes P+sod for those rows; each
    # chunk's sod goes straight to its output DMA. "v" DVE, "p" Pool.
    # Pool's ~2x TensorTensor handicap gives DVE ~2/3 of the rows; the
    # first chunks ramp up from 8 r so the DMA stream starts early
    # (smaller chunks would drop under the 512 B/partition contiguous
    # run threshold and pay a 2x DMA latency multiplier).
    CHUNKS = [
        [(0, 8, "v"), (8, 24, "v"), (24, 42, "v"), (42, 64, "p"),
         (64, 106, "v"), (106, 128, "p"), (128, 170, "v"),
         (170, 192, "p"), (192, 234, "v"), (234, 256, "p")],
        [(0, 42, "v"), (42, 64, "p"), (64, 106, "v"), (106, 128, "p"),
         (128, 170, "v"), (170, 192, "p"), (192, 234, "v"),
         (234, 256, "p")],
    ]

    with ExitStack() as ctx:
        tc = ctx.enter_context(tile.TileContext(nc))
        const = ctx.enter_context(tc.tile_pool(name="const", bufs=1))
        cst_sb = const.tile([IT, CW], f32)
        # unit 0 arrives in two pieces (biases + first r-ladder piece of
        # the skews first, a single producer for ACT's opening W instrs);
        # unit 1 as one piece.
        nc.sync.dma_start(cst_sb[:, 0 : 9 + 3 * RL], cst[:, 0 : 9 + 3 * RL])
        nc.sync.dma_start(cst_sb[:, 9 + 3 * RL : UW], cst[:, 9 + 3 * RL : UW])
        nc.sync.dma_start(cst_sb[:, UW : 2 * UW], cst[:, UW : 2 * UW])

        w01pool = ctx.enter_context(tc.tile_pool(name="w01", bufs=1))
        w2pool = ctx.enter_context(tc.tile_pool(name="w2", bufs=1))
        ppool = ctx.enter_context(tc.tile_pool(name="pp", bufs=1))
        sodpool = ctx.enter_context(tc.tile_pool(name="sod", bufs=1))

        # --- tiles for both units up front
        W01s, W2s, Pts, sods = [], [], [], []
        for u in range(UNITS):
            W01s.append(w01pool.tile([IT, 6, R], f32, name=f"w01_{u}"))
            W2s.append(w2pool.tile([IT, 3, R], f32, name=f"w2_{u}"))
            Pts.append(ppool.tile([IT, 9, R], f32, name=f"pt_{u}"))
            sods.append(sodpool.tile([IT, R, S], f32, name=f"sod_{u}"))

        # --- virtual-clock pre-scheduler: order each engine's queue by a
        # small event simulation using the measured cost model, so the
        # emitted order (which the tile scheduler largely keeps) has no
        # head-of-line stalls.
        SEM = 150.0
        DMA_READY = {(0, 0): 3250.0, (0, 1): 4350.0, (0, 2): 4350.0,
                     (0, 3): 4350.0, (1, 0): 5450.0, (1, 1): 5450.0}

        def act_cost(rl):
            return rl * 0.8333 + 185.0

        def dve_cost(n):
            return n * 1.0417 + 60.0

        def pool_cost(n):
            return n * 1.9841 + 95.0

        def pieces_of(u, r0, r1):
            return [pi for pi, (a, b) in enumerate(PIECES_U[u])
                    if r0 < b and r1 > a]

        def piece_off(u, pi):
            prev = sum(3 * (b - a) for a, b in PIECES_U[u][:pi])
            return u * UW + 9 + prev

        plan = []  # (t_start, seq, engine, kind, u, a, b, extra)
        seq = 0

        # ACT: fixed order; record W01/W2 completion per (u, piece)
        act_t = 0.0
        w01_done, w2_done = {}, {}
        for u in range(UNITS):
            for pi, (r0, r1) in enumerate(PIECES_U[u]):
                rl = r1 - r0
                for c in range(3):
                    for k in range(3):
                        t0 = max(act_t, DMA_READY[(u, pi)])
                        act_t = t0 + act_cost(rl)
                        plan.append((t0, seq, "act", "w", u, r0, r1,
                                     (c, k, pi)))
                        seq += 1
                        if c == 1 and k == 2:
                            w01_done[(u, pi)] = act_t
                        if c == 2 and k == 2:
                            w2_done[(u, pi)] = act_t

        def w01_ready(u, r0, r1):
            return max(w01_done[(u, pi)]
                       for pi in pieces_of(u, r0, r1)) + SEM

        def w2_ready(u, r0, r1):
            return max(w2_done[(u, pi)]
                       for pi in pieces_of(u, r0, r1)) + SEM

        def runs(u, owner):
            out, cur = [], None
            for q0, q1, own in CHUNKS[u]:
                if own != owner:
                    if cur:
                        out.append(cur)
                        cur = None
                    continue
                if cur and cur[1] == q0:
                    cur = (cur[0], q1)
                else:
                    if cur:
                        out.append(cur)
                    cur = (q0, q1)
            if cur:
                out.append(cur)
            return out

        # Both engines are FIFO production streams (no select stage);
        # each chunk's sod goes straight to its output DMA. DMAs are
        # emitted in simulated completion order so the SP queue never
        # head-of-line blocks.
        sod_done = []
        eng_t = {"dve": 0.0, "pool": 0.0}
        costf = {"dve": dve_cost, "pool": pool_cost}
        for u in range(UNITS):
            prun = {}
            for owner in ("v", "p"):
                for a, b in runs(u, owner):
                    prun[(owner, a)] = (a, b)
            for q0, q1, own in CHUNKS[u]:
                e = "dve" if own == "v" else "pool"
                if (own, q0) in prun:
                    a, b = prun[(own, q0)]
                    t0 = max(eng_t[e], w01_ready(u, a, b))
                    eng_t[e] = t0 + costf[e]((b - a) * 9)
                    plan.append((t0, seq, e, "P", u, a, b, None))
                    seq += 1
                t0 = max(eng_t[e], w2_ready(u, q0, q1))
                eng_t[e] = t0 + costf[e]((q1 - q0) * 27)
                plan.append((t0, seq, e, "sod", u, q0, q1, None))
                seq += 1
                sod_done.append((eng_t[e], u, q0, q1))
        sod_done.sort()
        for t, u, q0, q1 in sod_done:
            plan.append((t + SEM, seq, "sp", "dma", u, q0, q1, None))
            seq += 1

        # --- emit in global simulated start order
        plan.sort(key=lambda it: (it[0], it[1]))
        for t0, _s, engname, kind, u, a, b, extra in plan:
            W01, W2, Pt = W01s[u], W2s[u], Pts[u]
            sod = sods[u]
            if engname == "act":
                c, k, pi = extra
                r0, r1 = a, b
                rl = r1 - r0
                off = piece_off(u, pi)
                src_ap = cst_sb[:, off + c * rl : off + (c + 1) * rl]
                dst = (W01[:, 3 * c + k, r0:r1] if c < 2
                       else W2[:, k, r0:r1])
                b0 = u * UW
                nc.scalar.activation(
                    dst, src_ap, SQUARE,
                    bias=cst_sb[:, b0 + 3 * c + k : b0 + 3 * c + k + 1],
                    scale=1.0,
                )
                continue
            eng = nc.vector if engname == "dve" else nc.gpsimd
            if kind == "P":
                rc = b - a
                Pv = Pt[:].rearrange("p (x y) r -> p x y r", y=3)
                w0b = W01[:, 0:3, a:b].unsqueeze(2).broadcast_to(
                    [IT, 3, 3, rc])
                w1b = W01[:, 3:6, a:b].unsqueeze(1).broadcast_to(
                    [IT, 3, 3, rc])
                eng.tensor_tensor(Pv[:, :, :, a:b], w0b, w1b, ADD)
            elif kind == "sod":
                rc = b - a
                sv = sod[:].rearrange("p r (m c) -> p r m c", c=3)
                o = sv[:, a:b, :, :]
                pin = (Pt[:, :, a:b].rearrange("p m r -> p r m")
                       .unsqueeze(3).broadcast_to([IT, rc, 9, 3]))
                w2in = (W2[:, :, a:b].rearrange("p c r -> p r c")
                        .unsqueeze(2).broadcast_to([IT, rc, 9, 3]))
                eng.tensor_tensor(o, pin, w2in, ADD)
            else:  # dma
                nc.sync.dma_start(outv[u, :, a:b, :], sod[:, a:b, :])

    nc.compile()
    return nc


def _get_program():
    if "nc" not in _CACHE:
        _CACHE["nc"] = _build_program()
    return _CACHE["nc"]


def _prep_core_inputs(pos, tvals):
    """Per-core cst arrays. Core k: batch k//2, i-tiles 2*(k%2)+u.

    cst per-unit block: [bias(9) | c-major skews for r in [0,RL) |
    c-major skews for r in [RL,R)], where
      bias[3c+k]  = fl(pos[b, i0+p, c] + tvals[3c+k])
      skew[c][x]  = -pos[b, (i0+p+1+x) % N, c]
    """
    xs = np.arange(R)
    ps = np.arange(IT)
    tv = tvals.reshape(3, 3)
    in_maps = []
    for k in range(NCORES):
        b = k // 2
        cst = np.empty((IT, CW), np.float32)
        for u in range(UNITS):
            i0 = (2 * (k % 2) + u) * IT
            idx = (i0 + ps[:, None] + 1 + xs[None, :]) % N        # [IT, R]
            skew = -pos[b][idx].transpose(0, 2, 1)                 # [IT, 3, R]
            o = u * UW
            cst[:, o : o + 9] = (
                pos[b, i0 : i0 + IT, :, None] + tv[None, :, :]
            ).reshape(IT, 9)
            w = o + 9
            for a, bb in PIECES_U[u]:
                cst[:, w : w + 3 * (bb - a)] = skew[:, :, a:bb].reshape(
                    IT, -1
                )
                w += 3 * (bb - a)
        in_maps.append({"cst": cst})
    return in_maps


def _gather(results):
    out = np.zeros((B, N, N, S), np.float32)
    I = np.arange(N)
    J = (I[:, None] + np.arange(1, R + 1)[None, :]) % N            # [N, R]
    z = np.float32(0.0)
    for k in range(NCORES):
        b = k // 2
        ov = results[k]["outv"]                                    # [2,IT,R,S] f32
        for u in range(UNITS):
            i0 = (2 * (k % 2) + u) * IT
            sl = np.where(ov[u] <= np.float32(RC2), ov[u], z)
            Iu = I[i0 : i0 + IT, None]
            Ju = J[i0 : i0 + IT]
            out[b, Iu, Ju] = sl
            out[b, Ju, Iu] = sl[..., ::-1]
    return out


def _sample_ok(pos, tvals, results, n=256, tol=1e-2):
    """Spot-check random slab elements against a host recompute to
    catch transient device execution flakes (stale/garbled chunks)."""
    rng = np.random.RandomState(0)
    tv = tvals.reshape(3, 3)
    ks = rng.randint(0, NCORES, n)
    us = rng.randint(0, UNITS, n)
    ps = rng.randint(0, IT, n)
    xs = rng.randint(0, R, n)
    ss = rng.randint(0, S, n)
    for k, u, p, x, s in zip(ks, us, ps, xs, ss):
        b = k // 2
        i = (2 * (k % 2) + u) * IT + p
        j = (i + 1 + x) % N
        k0, k1, k2 = s // 9, (s // 3) % 3, s % 3
        w = [np.float32(np.float32(-pos[b, j, c]
             + np.float32(pos[b, i, c] + tv[c, kk])) ** 2)
             for c, kk in ((0, k0), (1, k1), (2, k2))]
        ref = np.float32(np.float32(w[0] + w[1]) + w[2])
        got = results[k]["outv"][u, p, x, s]
        if abs(float(got) - float(ref)) > tol * max(1.0, abs(float(ref))):
            return False
    return True


def _analyze_shifts(cel_mat, sft_cel):
    """Return tvals[9] f32 if inputs have the standard structure
    (diagonal cell, sft = meshgrid(-1..1)^3), else None.

    tvals[3*c + k] is the k-th shift value on axis c, ordered so that
    s = 9*k0 + 3*k1 + k2 indexes sft_xyz[s] = (t0[k0], t1[k1], t2[k2]).
    """
    r = np.arange(-1, 2)
    expect = np.stack(np.meshgrid(r, r, r, indexing="ij"), axis=-1).reshape(-1, 3)
    if sft_cel.shape != (27, 3) or not np.array_equal(sft_cel, expect):
        return None
    cel0 = cel_mat[0]
    if not np.all(cel_mat == cel0[None]):
        return None
    if np.any(cel0 != np.diag(np.diag(cel0))):
        return None
    diag = np.diag(cel0).astype(np.float32)
    # sft_xyz[s, c] = sum_d sft[s,d] * cel[d,c] = sft[s,c] * diag[c] exactly
    tvals = np.empty(9, np.float32)
    for c in range(3):
        for k in range(3):
            tvals[3 * c + k] = np.float32(np.float32(k - 1) * diag[c])
    return tvals


def _reference_fallback(pos_xyz, cel_mat, pbc, ent, sft_cel):
    """Plain numpy mirror of the reference (for non-standard inputs only)."""
    sft_xyz = np.einsum(
        "sd,bde->bse", sft_cel.astype(cel_mat.dtype), cel_mat
    )
    vec = (
        pos_xyz[:, :, None, None, :]
        - pos_xyz[:, None, :, None, :]
        + sft_xyz[:, None, None, :, :]
    )
    sod = np.sum(vec * vec, axis=-1)
    n = pos_xyz.shape[1]
    eye = np.eye(n, dtype=bool)
    zero_sft = np.all(sft_cel == 0, axis=-1)
    self_pair = eye[None, :, :, None] & zero_sft[None, None, None, :]
    val = ent[:, :, None, None] & ent[:, None, :, None]
    mask = (sod <= RC2) & val & ~self_pair
    out = np.where(mask, sod, np.zeros((), sod.dtype))
    return out, mask


def kernel(pos_xyz, cel_mat, pbc, ent, sft_cel):
    pos_xyz = np.asarray(pos_xyz)
    cel_mat = np.asarray(cel_mat)
    pbc = np.asarray(pbc)
    ent = np.asarray(ent)
    sft_cel = np.asarray(sft_cel)

    tvals = None
    if pos_xyz.shape == (B, N, 3) and pos_xyz.dtype == np.float32:
        tvals = _analyze_shifts(cel_mat, sft_cel)
    if tvals is None:
        return _reference_fallback(pos_xyz, cel_mat, pbc, ent, sft_cel)

    from concourse.bass_utils import run_bass_kernel_spmd

    nc = _get_program()
    in_maps = _prep_core_inputs(pos_xyz, tvals)
    trace = os.environ.get("BENCH_TRACE", "") == "1"
    for attempt in range(2):
        res = run_bass_kernel_spmd(
            nc, in_maps, core_ids=list(range(NCORES)), trace=trace
        )
        if attempt == 0 and not _sample_ok(pos_xyz, tvals, res.results):
            continue  # transient device flake: retry once
        break
    _CACHE["last_results"] = res
    out = _gather(res.results)

    # The select is decided on-device from the exact f32 sod; shipped
    # values are fp16-rounded, never crossing zero, so out > 0 is
    # exactly the reference mask (self pairs land at out == 0).
    mask = out > 0
    if not ent.all():
        val = ent[:, :, None, None] & ent[:, None, :, None]
        mask &= val[..., None]
        out *= mask
    return out, mask
